# revision 21
# baseline (speedup 1.0000x reference)
"""Trainium2 Bass kernel for nn_Attention_84473416778449.

Reference computation (B=2, S=2048, D=1024, H=16, HD=64, fp32):
    q/k/v = x @ w{q,k,v}.T ; RoPE(q, k) ; causal softmax attention ; out @ wo.T

Sharding: 8 cores = (batch 2) x (head-group 4). Each core computes 4 heads of
one batch end-to-end and a partial output projection over its 256 channels;
the host sums the 4 partials per batch (fp16 partials, fp32 accumulate).

Attention runs as two head-PAIRS per 512-row block qb. Per (pair, kt) step the
two heads' score matmuls (K=64) issue back-to-back at tile_position (0,0) /
(64,0) — concurrent on disjoint PE row groups — into one 2-bank PSUM tile
[128, 1024]; a single ACTIVATE exps both banks; causal masking touches only
the first 128 columns of diagonal tiles (one paired gpsimd affine_select);
two PV matmuls (K=128) accumulate into a 2-bank po tile whose row 64 carries
softmax denominators (ones-column in v). Normalization: ACT copies the
denominator row to SBUF fp16, two col-tiled K=1 broadcast matmuls fan it to
128 partitions, a PSUM-side approx reciprocal and two DVE mults produce the
normalized transposed attention output.

The QKV projections (+RoPE) for block sb+1 and the output projections for
block sb-1 are emitted as GENERATORS that yield after every matmul; the
attention kt loop advances them a fractional quota per step, so the PE always
has dense independent work while ACT exps the current score tile. This keeps
HAM at K=8/8 and hides the exp latency. The final (projection-free) block
additionally issues dependency-free dummy LDWEIGHTS to hold PE activity up.

Host passes x/weights pre-transposed, pre-cast to fp16, and pre-swizzled into
the exact SBUF layouts, so every load is one fully-contiguous DMA (weights on
the scalar HWDGE queue, x blocks on sync); the first projection matmul issues
~2.5us in and HAM warms during the loads. Output is stored fp16.
"""
import sys

if "/opt/trn_rl_repo" not in sys.path:
    sys.path.insert(0, "/opt/trn_rl_repo")

import numpy as np

import concourse.bass as bass
import concourse.mybir as mybir
import concourse.tile as tile
from concourse import bacc
from concourse.bass_utils import run_bass_kernel_spmd

B, S, D, H, HD = 2, 2048, 1024, 16, 64
NCORES = 8
GROUPS = 4            # head groups
GH = H // GROUPS      # heads per group = 4
GC = GH * HD          # channels per group = 256
KT = D // 128         # 8 k-tiles over D
ST = S // 128         # 16 s-tiles
QB = 4                # sq blocks of 512
QW = S // QB          # 512
VW = GH * (HD + 1)    # 260: v tile payload columns
XBW = KT * QW         # 4096: columns of one x block tile

f32 = mybir.dt.float32
MMDT = mybir.dt.float16   # matmul-operand dtype
Exp = mybir.ActivationFunctionType.Exp
Copy = mybir.ActivationFunctionType.Copy
Mult = mybir.AluOpType.mult
Add = mybir.AluOpType.add

_cache = {}


def _build():
    nc = bacc.Bacc("TRN2", num_devices=NCORES)

    xTd = nc.dram_tensor("xTd", [128, QB * XBW], MMDT,
                         kind="ExternalInput").ap()
    wqT = nc.dram_tensor("wqT", [128, KT * GC], MMDT,
                         kind="ExternalInput").ap()
    wkT = nc.dram_tensor("wkT", [128, KT * GC], MMDT,
                         kind="ExternalInput").ap()
    wvT = nc.dram_tensor("wvT", [128, KT * GC], MMDT,
                         kind="ExternalInput").ap()
    woT = nc.dram_tensor("woT", [GC, D], MMDT, kind="ExternalInput").ap()
    cs2 = nc.dram_tensor("cs2", [128, S], MMDT, kind="ExternalInput").ap()
    sn2 = nc.dram_tensor("sn2", [128, S], MMDT, kind="ExternalInput").ap()
    out = nc.dram_tensor("out", [S, D], MMDT, kind="ExternalOutput").ap()

    with tile.TileContext(nc) as tc:
        with tc.tile_pool(name="persist", bufs=1) as pp, \
             tc.tile_pool(name="rope", bufs=3) as rp, \
             tc.tile_pool(name="probs", bufs=3) as wp, \
             tc.tile_pool(name="outsb", bufs=3) as op_, \
             tc.tile_pool(name="small", bufs=1) as sp:

            # ---- persistent SBUF tiles -------------------------------------
            xb = [pp.tile([128, XBW], MMDT, tag=f"xb{cb}", name=f"xb{cb}")
                  for cb in range(QB)]
            w_sb = {}
            for nm in ("wq", "wk", "wv"):
                w_sb[nm] = pp.tile([128, KT * GC], MMDT, tag=f"w{nm}",
                                   name=f"w{nm}")
            wo_s = [pp.tile([128, D], MMDT, tag=f"wo{kt}", name=f"wo{kt}")
                    for kt in range(2)]
            cs_sb = pp.tile([128, S], MMDT, tag="cs")
            sn_sb = pp.tile([128, S], MMDT, tag="sn")
            qT = [[pp.tile([128, QW], MMDT, tag=f"qT{i}_{b}",
                           name=f"qT{i}_{b}") for b in range(QB)]
                  for i in range(2)]
            kTt = [[pp.tile([128, QW], MMDT, tag=f"kT{i}_{b}",
                            name=f"kT{i}_{b}") for b in range(QB)]
                   for i in range(2)]
            attnT = [[pp.tile([128, QW], MMDT, tag=f"aT{i}_{b}",
                              name=f"aT{i}_{b}") for b in range(QB)]
                     for i in range(2)]
            v_sb = [pp.tile([128, VW + 64], MMDT, tag=f"v{i}",
                            name=f"v{i}") for i in range(ST)]

            # ---- DMA issue: weights on scalar HWDGE, x + cs/sn on sync -----
            def load_cssn(sb):
                cols = slice(sb * QW, (sb + 1) * QW)
                nc.sync.dma_start(cs_sb[:, cols], cs2[:, cols])
                nc.sync.dma_start(sn_sb[:, cols], sn2[:, cols])

            nc.sync.dma_start(xb[0][:, 0:XBW // 2], xTd[:, 0:XBW // 2])
            nc.scalar.dma_start(w_sb["wq"][:], wqT[:])
            nc.sync.dma_start(xb[0][:, XBW // 2:XBW],
                              xTd[:, XBW // 2:XBW])
            nc.scalar.dma_start(w_sb["wk"][:], wkT[:])
            load_cssn(0)
            nc.scalar.dma_start(w_sb["wv"][:], wvT[:])
            nc.sync.dma_start(xb[1][:], xTd[:, XBW:2 * XBW])
            load_cssn(1)
            for kt in range(2):
                nc.scalar.dma_start(wo_s[kt][:],
                                    woT[kt * 128:(kt + 1) * 128, :])

            def load_x_block(cb):
                nc.sync.dma_start(xb[cb][:],
                                  xTd[:, cb * XBW:(cb + 1) * XBW])
                load_cssn(cb)

            # ---- constants (ones/zeros first: they feed HAM warm-up) ------
            ones16 = pp.tile([1, 128], MMDT, tag="ones16")
            nc.gpsimd.memset(ones16[:], 1.0)
            zeros16 = pp.tile([1, QW], MMDT, tag="zeros16")
            nc.gpsimd.memset(zeros16[:], 0.0)
            cscratch = pp.tile([128, 128], f32, tag="cscratch")
            nc.gpsimd.memset(cscratch[:], 0.0)
            for blk in range(2):
                sub = cscratch[blk * 64:(blk + 1) * 64,
                               blk * 64:(blk + 1) * 64]
                nc.gpsimd.affine_select(   # -1 where p - f == 32
                    out=sub, in_=sub, pattern=[[-1, 64]], base=-32,
                    channel_multiplier=1,
                    compare_op=mybir.AluOpType.not_equal, fill=-1.0)
                nc.gpsimd.affine_select(   # +1 where f - p == 32
                    out=sub, in_=sub, pattern=[[1, 64]], base=-32,
                    channel_multiplier=-1,
                    compare_op=mybir.AluOpType.not_equal, fill=1.0)
            rt2 = pp.tile([128, 128], MMDT, tag="rt2")
            nc.vector.tensor_copy(rt2[:], cscratch[:])
            for st in range(ST):
                vt = v_sb[st]
                vhe = vt[:, 0:VW].rearrange("p (h e) -> p h e", e=HD + 1)
                nc.gpsimd.memset(vt[:, VW:VW + 64], 0.0)
                nc.gpsimd.memset(vhe[:, :, HD:HD + 1], 1.0)

            with tc.tile_pool(name="psS", bufs=2, space="PSUM") as psS, \
                 tc.tile_pool(name="psO", bufs=1, space="PSUM") as psO, \
                 tc.tile_pool(name="psP", bufs=2, space="PSUM") as psP:

                def xsl(sb, kt, off=0, width=QW):
                    return xb[sb][:, kt * QW + off:kt * QW + off + width]

                def gen_qk_proj(nm, dst, hp, sb):
                    w_src = w_sb[nm]
                    pq = psP.tile([128, QW], f32, tag="proj",
                                  name=f"pq_{nm}_{hp}_{sb}")
                    for kt in range(KT):
                        nc.tensor.matmul(
                            pq[:],
                            w_src[:, kt * GC + hp * 128:
                                  kt * GC + hp * 128 + 128],
                            xsl(sb, kt),
                            start=(kt == 0), stop=(kt == KT - 1))
                        yield
                    cols = slice(sb * QW, (sb + 1) * QW)
                    tcs = rp.tile([128, QW], MMDT, tag="tcs")
                    nc.vector.tensor_tensor(
                        out=tcs[:], in0=pq[:], in1=cs_sb[:, cols], op=Mult)
                    tsn = rp.tile([128, QW], MMDT, tag="tsn")
                    nc.vector.tensor_tensor(
                        out=tsn[:], in0=pq[:], in1=sn_sb[:, cols], op=Mult)
                    pr = psP.tile([128, QW], f32, tag="proj",
                                  name=f"pr_{nm}_{hp}_{sb}")
                    nc.tensor.matmul(pr[:], rt2[:], tsn[:],
                                     start=True, stop=True)
                    yield
                    nc.vector.tensor_tensor(
                        out=dst[hp][sb][:], in0=pr[:], in1=tcs[:], op=Add)

                def gen_v(st):
                    pv = psP.tile([128, QW], f32, tag="proj",
                                  name=f"pv_{st}")
                    for kt in range(KT):
                        nc.tensor.matmul(
                            pv[:, 0:GC],
                            xsl(st // 4, kt, (st % 4) * 128, 128),
                            w_sb["wv"][:, kt * GC:(kt + 1) * GC],
                            start=(kt == 0), stop=(kt == KT - 1))
                        yield
                    vt = v_sb[st]
                    vhe = vt[:, 0:VW].rearrange("p (h e) -> p h e", e=HD + 1)
                    nc.vector.tensor_copy(
                        vhe[:, :, 0:HD],
                        pv[:, 0:GC].rearrange("p (h d) -> p h d", d=HD))

                out_tog = [0]

                def gen_out(st, db):
                    pc = psP.tile([128, QW], f32, tag="proj",
                                  name=f"pc_{st}_{db}")
                    for kt in range(2):
                        nc.tensor.matmul(
                            pc[:],
                            attnT[kt][st // 4][:, (st % 4) * 128:
                                               (st % 4) * 128 + 128],
                            wo_s[kt][:, db * QW:(db + 1) * QW],
                            start=(kt == 0), stop=(kt == 1))
                        yield
                    ob = op_.tile([128, QW], MMDT, tag="outsb")
                    dst = out[st * 128:(st + 1) * 128, db * QW:(db + 1) * QW]
                    if out_tog[0] % 2 == 0:
                        nc.vector.tensor_copy(ob[:], pc[:])
                        nc.sync.dma_start(dst, ob[:])
                    else:
                        nc.scalar.activation(ob[:], pc[:], Copy)
                        nc.scalar.dma_start(dst, ob[:])
                    out_tog[0] += 1

                def normalize(hp, qb, po, tail_out=None):
                    den16 = sp.tile([1, 2 * QW], MMDT, tag="den")
                    nc.scalar.activation(den16[:], po[HD:HD + 1, 0:2 * QW],
                                         Copy)
                    if tail_out is None:
                        # copy po values to SBUF so po frees for the next
                        # pair as soon as both copies land (not after the
                        # whole reciprocal chain)
                        vo = sp.tile([128, QW], f32, tag="vo")
                        nc.scalar.activation(vo[0:64, :], po[0:HD, 0:QW],
                                             Copy)
                        nc.scalar.activation(vo[64:128, :],
                                             po[0:HD, QW:2 * QW], Copy)
                    recb_ps = psP.tile([128, QW], f32, tag="proj",
                                       name=f"rb_{hp}_{qb}")
                    nc.tensor.matmul(recb_ps[0:64, :], ones16[0:1, 0:64],
                                     den16[0:1, 0:QW],
                                     start=True, stop=True,
                                     tile_position=(0, 0))
                    nc.tensor.matmul(recb_ps[64:128, :], ones16[0:1, 0:64],
                                     den16[0:1, QW:2 * QW],
                                     start=True, stop=True,
                                     tile_position=(0, 64))
                    recb = sp.tile([128, QW], f32, tag="recb")
                    rsc = sp.tile([128, QW], f32, tag="rscr")
                    nc.vector.reciprocal_approx_accurate(
                        out=recb[:], in_=recb_ps[:], scratch=rsc[:])
                    if tail_out is None:
                        nc.vector.tensor_tensor(
                            out=attnT[hp][qb][0:64, :],
                            in0=vo[0:64, :], in1=recb[0:64, :], op=Mult)
                        nc.vector.tensor_tensor(
                            out=attnT[hp][qb][64:128, :],
                            in0=vo[64:128, :], in1=recb[64:128, :],
                            op=Mult)
                    else:
                        # final pair: write attnT in 128-col chunks and
                        # launch the tail output projections per chunk
                        for ck in range(4):
                            cs_ = slice(ck * 128, (ck + 1) * 128)
                            nc.vector.tensor_tensor(
                                out=attnT[hp][qb][0:64, cs_],
                                in0=po[0:HD, ck * 128:(ck + 1) * 128],
                                in1=recb[0:64, cs_], op=Mult)
                            nc.vector.tensor_tensor(
                                out=attnT[hp][qb][64:128, cs_],
                                in0=po[0:HD, QW + ck * 128:
                                       QW + (ck + 1) * 128],
                                in1=recb[64:128, cs_], op=Mult)
                            for g in tail_out[ck]:
                                for _ in g:
                                    pass

                def attn_block(qb, fillers, nmicro, tail_out=None):
                    nsk = 4 * (qb + 1)
                    nsteps = 2 * nsk
                    acc = [0.0]

                    def pop(n):
                        while n > 0 and fillers:
                            try:
                                next(fillers[0])
                            except StopIteration:
                                fillers.pop(0)
                            n -= 1

                    def emit_pv(po, prts, h0, h1, kt, nsk):
                        c0 = max(0, kt * 128 - qb * QW)
                        cw = QW - c0
                        prt = prts[kt]
                        nc.tensor.matmul(
                            po[:, c0:QW],
                            v_sb[kt][:, h0 * (HD + 1):h0 * (HD + 1) + 128],
                            prt[:, 0:cw],
                            start=(kt == 0), stop=(kt == nsk - 1))
                        nc.tensor.matmul(
                            po[:, QW + c0:2 * QW],
                            v_sb[kt][:, h1 * (HD + 1):h1 * (HD + 1) + 128],
                            prt[:, QW:QW + cw],
                            start=(kt == 0), stop=(kt == nsk - 1))

                    for hp in range(2):
                        h0, h1 = 2 * hp, 2 * hp + 1
                        po = psO.tile([128, 2 * QW], f32, tag="pvacc",
                                      name=f"po_{hp}_{qb}")
                        prts = {}
                        for kt in range(nsk):
                            c0 = max(0, kt * 128 - qb * QW)
                            cw = QW - c0
                            sc = psS.tile([128, 2 * QW], f32, tag="score",
                                          name=f"sc_{hp}_{qb}_{kt}")
                            nc.tensor.matmul(
                                sc[:, 0:cw],
                                kTt[hp][kt // 4][0:64, (kt % 4) * 128:
                                                 (kt % 4) * 128 + 128],
                                qT[hp][qb][0:64, c0:QW],
                                start=True, stop=True,
                                tile_position=(0, 0))
                            nc.tensor.matmul(
                                sc[:, QW:QW + cw],
                                kTt[hp][kt // 4][64:128, (kt % 4) * 128:
                                                 (kt % 4) * 128 + 128],
                                qT[hp][qb][64:128, c0:QW],
                                start=True, stop=True,
                                tile_position=(64, 0))
                            prt = wp.tile([128, 2 * QW], MMDT, tag="probs",
                                          name=f"pr_{hp}_{qb}_{kt}")
                            prts[kt] = prt
                            diag = kt >= nsk - 4
                            if diag:
                                sc3 = sc[:].rearrange(
                                    "p (h c) -> p h c", c=QW)[:, :, 0:cw]
                                pr3 = prt[:].rearrange(
                                    "p (h c) -> p h c", c=QW)[:, :, 0:cw]
                                nc.scalar.activation(pr3, sc3, Exp,
                                                     scale=0.125)
                                pr128 = prt[:].rearrange(
                                    "p (h c) -> p h c", c=QW)[:, :, 0:128]
                                nc.gpsimd.affine_select(
                                    out=pr128, in_=pr128,
                                    pattern=[[0, 2], [1, 128]], base=0,
                                    channel_multiplier=-1,
                                    compare_op=mybir.AluOpType.is_ge,
                                    fill=0.0)
                            else:
                                nc.scalar.activation(prt[:], sc[:], Exp,
                                                     scale=0.125)
                            # filler quota while ACT runs the exp
                            acc[0] += nmicro / nsteps
                            nq = int(acc[0])
                            acc[0] -= nq
                            pop(nq)
                            if not fillers and kt >= 2:
                                # keep HAM busy: accumulate zeros onto po's
                                # garbage rows (96:127 are unused v overlap)
                                nc.tensor.matmul(
                                    po[96:128, 0:QW], ones16[0:1, 0:32],
                                    zeros16[0:1, 0:QW],
                                    start=False, stop=False,
                                    tile_position=(0, 96),
                                    skip_group_check=True)
                            if kt > 0:
                                emit_pv(po, prts, h0, h1, kt - 1, nsk)
                        emit_pv(po, prts, h0, h1, nsk - 1, nsk)
                        normalize(hp, qb, po,
                                  tail_out if qb == QB - 1 and hp == 1
                                  else None)
                    while fillers:
                        pop(1)

                # ---- HAM warm-up while the first loads are in flight -------
                for i in range(40):
                    wt = psP.tile([128, QW], f32, tag="proj",
                                  name=f"warm{i}")
                    nc.tensor.matmul(wt[:], ones16[0:1, 0:128],
                                     zeros16[0:1, 0:QW],
                                     start=True, stop=True)

                # ---- block 0 projections (overlap the initial loads) -------
                for nm, dst in (("wq", qT), ("wk", kTt)):
                    for hp in range(2):
                        for _ in gen_qk_proj(nm, dst, hp, 0):
                            pass
                for st in range(4):
                    for _ in gen_v(st):
                        pass

                def proj_units(sb):
                    u = []
                    for hp in range(2):
                        u.append(gen_qk_proj("wq", qT, hp, sb))
                        u.append(gen_qk_proj("wk", kTt, hp, sb))
                    for st in range(4 * sb, 4 * sb + 4):
                        u.append(gen_v(st))
                    return u, 4 * 10 + 4 * 9

                def out_units(sb):
                    return ([gen_out(st, db)
                             for st in range(4 * sb, 4 * sb + 4)
                             for db in range(2)], 8 * 3)

                for sb in range(QB):
                    if sb + 2 < QB:
                        load_x_block(sb + 2)
                    fillers, nmicro = [], 0
                    if sb > 0:
                        ou, n = out_units(sb - 1)
                        fillers += ou
                        nmicro += n
                    if sb + 1 < QB:
                        pu, n = proj_units(sb + 1)
                        # interleave out units and proj units
                        mixed = []
                        for i in range(max(len(pu), len(fillers))):
                            if i < len(fillers):
                                mixed.append(fillers[i])
                            if i < len(pu):
                                mixed.append(pu[i])
                        fillers = mixed
                        nmicro += n
                    tail = None
                    if sb == QB - 1:
                        out_tog[0] = 0
                        tail = [[gen_out(12 + ck, 0), gen_out(12 + ck, 1)]
                                for ck in range(4)]
                    attn_block(sb, fillers, nmicro, tail)

    nc.compile()
    return nc


def _shard_inputs(x, cos, sin, wq, wk, wv, wo):
    x = np.ascontiguousarray(x, dtype=np.float32)
    cosT = np.ascontiguousarray(cos.reshape(S, HD).T, dtype=np.float32)
    sinT = np.ascontiguousarray(sin.reshape(S, HD).T, dtype=np.float32)
    cs2 = np.ascontiguousarray(np.concatenate([cosT, cosT], axis=0)
                               .astype(np.float16))
    sn2 = np.ascontiguousarray(np.concatenate([sinT, sinT], axis=0)
                               .astype(np.float16))
    f16 = np.float16

    def wdev(w):
        # [D, GC] -> [128, KT*GC] with kt chunks on columns
        return np.ascontiguousarray(
            w.reshape(KT, 128, GC).transpose(1, 0, 2).reshape(128, KT * GC))

    in_maps = []
    for c in range(NCORES):
        b, g = c // GROUPS, c % GROUPS
        rows = slice(g * GC, (g + 1) * GC)
        xT = x[b].T.astype(f16)            # [D, S]
        xdev = np.ascontiguousarray(
            xT.reshape(KT, 128, QB, QW).transpose(1, 2, 0, 3)
            .reshape(128, QB * XBW))
        in_maps.append({
            "xTd": xdev,
            "wqT": wdev(np.asarray(wq, f16)[rows, :].T),
            "wkT": wdev(np.asarray(wk, f16)[rows, :].T),
            "wvT": wdev(np.asarray(wv, f16)[rows, :].T),
            "woT": np.ascontiguousarray(np.asarray(wo, f16)[:, rows].T),
            "cs2": cs2,
            "sn2": sn2,
        })
    return in_maps


def _run(inputs, trace=False, trace_kwargs=None):
    if "nc" not in _cache:
        _cache["nc"] = _build()
    nc = _cache["nc"]
    in_maps = _shard_inputs(
        inputs["x"], inputs["cos"], inputs["sin"],
        inputs["wq"], inputs["wk"], inputs["wv"], inputs["wo"])
    res = run_bass_kernel_spmd(
        nc, in_maps, list(range(NCORES)), trace=trace,
        **(trace_kwargs or {}))
    full = np.zeros((B, S, D), dtype=np.float32)
    for c in range(NCORES):
        full[c // GROUPS] += res.results[c]["out"].astype(np.float32)
    return full, res


def kernel(**inputs):
    full, _ = _run(inputs, trace=False)
    return full


# revision 22
# speedup vs baseline: 1.0725x; 1.0725x over previous
"""Trainium2 Bass kernel for nn_Attention_84473416778449.

Reference computation (B=2, S=2048, D=1024, H=16, HD=64, fp32):
    q/k/v = x @ w{q,k,v}.T ; RoPE(q, k) ; causal softmax attention ; out @ wo.T

Sharding: 8 cores = (batch 2) x (head-group 4). Each core computes 4 heads of
one batch end-to-end and a partial output projection over its 256 channels;
the host sums the 4 partials per batch (fp16 partials, fp32 accumulate).

Attention runs as two head-PAIRS per 512-row block qb. Per (pair, kt) step the
two heads' score matmuls (K=64) issue back-to-back at tile_position (0,0) /
(64,0) — concurrent on disjoint PE row groups — into one 2-bank PSUM tile
[128, 1024]; a single ACTIVATE exps both banks; causal masking touches only
the first 128 columns of diagonal tiles (one paired gpsimd affine_select);
two PV matmuls (K=128) accumulate into a 2-bank po tile whose row 64 carries
softmax denominators (ones-column in v). Normalization: ACT copies the
denominator row to SBUF fp16, two col-tiled K=1 broadcast matmuls fan it to
128 partitions, a PSUM-side approx reciprocal and two DVE mults produce the
normalized transposed attention output.

The QKV projections (+RoPE) for block sb+1 and the output projections for
block sb-1 are emitted as GENERATORS that yield after every matmul; the
attention kt loop advances them a fractional quota per step, so the PE always
has dense independent work while ACT exps the current score tile. This keeps
HAM at K=8/8 and hides the exp latency. The final (projection-free) block
additionally issues dependency-free dummy LDWEIGHTS to hold PE activity up.

Host passes x/weights pre-transposed, pre-cast to fp16, and pre-swizzled into
the exact SBUF layouts, so every load is one fully-contiguous DMA (weights on
the scalar HWDGE queue, x blocks on sync); the first projection matmul issues
~2.5us in and HAM warms during the loads. Output is stored fp16.
"""
import sys

if "/opt/trn_rl_repo" not in sys.path:
    sys.path.insert(0, "/opt/trn_rl_repo")

import numpy as np

import concourse.bass as bass
import concourse.mybir as mybir
import concourse.tile as tile
from concourse import bacc
from concourse.bass_utils import run_bass_kernel_spmd

B, S, D, H, HD = 2, 2048, 1024, 16, 64
NCORES = 8
GROUPS = 4            # head groups
GH = H // GROUPS      # heads per group = 4
GC = GH * HD          # channels per group = 256
KT = D // 128         # 8 k-tiles over D
ST = S // 128         # 16 s-tiles
QB = 4                # sq blocks of 512
QW = S // QB          # 512
VW = GH * (HD + 1)    # 260: v tile payload columns
XBW = KT * QW         # 4096: columns of one x block tile

f32 = mybir.dt.float32
MMDT = mybir.dt.float16   # matmul-operand dtype
Exp = mybir.ActivationFunctionType.Exp
Copy = mybir.ActivationFunctionType.Copy
Mult = mybir.AluOpType.mult
Add = mybir.AluOpType.add

_cache = {}


def _build():
    nc = bacc.Bacc("TRN2", num_devices=NCORES)

    xTd = nc.dram_tensor("xTd", [128, QB * XBW], MMDT,
                         kind="ExternalInput").ap()
    wqT = nc.dram_tensor("wqT", [128, KT * GC], MMDT,
                         kind="ExternalInput").ap()
    wkT = nc.dram_tensor("wkT", [128, KT * GC], MMDT,
                         kind="ExternalInput").ap()
    wvT = nc.dram_tensor("wvT", [128, KT * GC], MMDT,
                         kind="ExternalInput").ap()
    woT = nc.dram_tensor("woT", [GC, D], MMDT, kind="ExternalInput").ap()
    cs2 = nc.dram_tensor("cs2", [128, S], MMDT, kind="ExternalInput").ap()
    sn2 = nc.dram_tensor("sn2", [128, S], MMDT, kind="ExternalInput").ap()
    out = nc.dram_tensor("out", [S, D], MMDT, kind="ExternalOutput").ap()

    with tile.TileContext(nc) as tc:
        with tc.tile_pool(name="persist", bufs=1) as pp, \
             tc.tile_pool(name="rope", bufs=3) as rp, \
             tc.tile_pool(name="probs", bufs=3) as wp, \
             tc.tile_pool(name="outsb", bufs=3) as op_, \
             tc.tile_pool(name="small", bufs=1) as sp:

            # ---- persistent SBUF tiles -------------------------------------
            xb = [pp.tile([128, XBW], MMDT, tag=f"xb{cb}", name=f"xb{cb}")
                  for cb in range(QB)]
            w_sb = {}
            for nm in ("wq", "wk", "wv"):
                w_sb[nm] = pp.tile([128, KT * GC], MMDT, tag=f"w{nm}",
                                   name=f"w{nm}")
            wo_s = [pp.tile([128, D], MMDT, tag=f"wo{kt}", name=f"wo{kt}")
                    for kt in range(2)]
            cs_sb = pp.tile([128, S], MMDT, tag="cs")
            sn_sb = pp.tile([128, S], MMDT, tag="sn")
            qT = [[pp.tile([128, QW], MMDT, tag=f"qT{i}_{b}",
                           name=f"qT{i}_{b}") for b in range(QB)]
                  for i in range(2)]
            kTt = [[pp.tile([128, QW], MMDT, tag=f"kT{i}_{b}",
                            name=f"kT{i}_{b}") for b in range(QB)]
                   for i in range(2)]
            attnT = [[pp.tile([128, QW], MMDT, tag=f"aT{i}_{b}",
                              name=f"aT{i}_{b}") for b in range(QB)]
                     for i in range(2)]
            v_sb = [pp.tile([128, VW + 64], MMDT, tag=f"v{i}",
                            name=f"v{i}") for i in range(ST)]

            # ---- DMA issue: weights on scalar HWDGE, x + cs/sn on sync -----
            def load_cssn(sb):
                cols = slice(sb * QW, (sb + 1) * QW)
                nc.sync.dma_start(cs_sb[:, cols], cs2[:, cols])
                nc.sync.dma_start(sn_sb[:, cols], sn2[:, cols])

            nc.sync.dma_start(xb[0][:, 0:XBW // 2], xTd[:, 0:XBW // 2])
            nc.scalar.dma_start(w_sb["wq"][:], wqT[:])
            nc.sync.dma_start(xb[0][:, XBW // 2:XBW],
                              xTd[:, XBW // 2:XBW])
            nc.scalar.dma_start(w_sb["wk"][:], wkT[:])
            load_cssn(0)
            nc.scalar.dma_start(w_sb["wv"][:], wvT[:])
            nc.sync.dma_start(xb[1][:], xTd[:, XBW:2 * XBW])
            load_cssn(1)
            for kt in range(2):
                nc.scalar.dma_start(wo_s[kt][:],
                                    woT[kt * 128:(kt + 1) * 128, :])

            def load_x_block(cb):
                nc.sync.dma_start(xb[cb][:],
                                  xTd[:, cb * XBW:(cb + 1) * XBW])
                load_cssn(cb)

            # ---- constants -------------------------------------------------
            cscratch = pp.tile([128, 128], f32, tag="cscratch")
            nc.gpsimd.memset(cscratch[:], 0.0)
            for blk in range(2):
                sub = cscratch[blk * 64:(blk + 1) * 64,
                               blk * 64:(blk + 1) * 64]
                nc.gpsimd.affine_select(   # -1 where p - f == 32
                    out=sub, in_=sub, pattern=[[-1, 64]], base=-32,
                    channel_multiplier=1,
                    compare_op=mybir.AluOpType.not_equal, fill=-1.0)
                nc.gpsimd.affine_select(   # +1 where f - p == 32
                    out=sub, in_=sub, pattern=[[1, 64]], base=-32,
                    channel_multiplier=-1,
                    compare_op=mybir.AluOpType.not_equal, fill=1.0)
            rt2 = pp.tile([128, 128], MMDT, tag="rt2")
            nc.vector.tensor_copy(rt2[:], cscratch[:])
            ones16 = pp.tile([1, 128], MMDT, tag="ones16")
            nc.gpsimd.memset(ones16[:], 1.0)
            zeros16 = pp.tile([1, QW], MMDT, tag="zeros16")
            nc.gpsimd.memset(zeros16[:], 0.0)
            for st in range(ST):
                vt = v_sb[st]
                vhe = vt[:, 0:VW].rearrange("p (h e) -> p h e", e=HD + 1)
                nc.gpsimd.memset(vt[:, VW:VW + 64], 0.0)
                nc.gpsimd.memset(vhe[:, :, HD:HD + 1], 1.0)

            with tc.tile_pool(name="psS", bufs=2, space="PSUM") as psS, \
                 tc.tile_pool(name="psO", bufs=1, space="PSUM") as psO, \
                 tc.tile_pool(name="psP", bufs=2, space="PSUM") as psP:

                def xsl(sb, kt, off=0, width=QW):
                    return xb[sb][:, kt * QW + off:kt * QW + off + width]

                def gen_qk_proj(nm, dst, hp, sb):
                    w_src = w_sb[nm]
                    pq = psP.tile([128, QW], f32, tag="proj",
                                  name=f"pq_{nm}_{hp}_{sb}")
                    for kt in range(KT):
                        nc.tensor.matmul(
                            pq[:],
                            w_src[:, kt * GC + hp * 128:
                                  kt * GC + hp * 128 + 128],
                            xsl(sb, kt),
                            start=(kt == 0), stop=(kt == KT - 1))
                        yield
                    cols = slice(sb * QW, (sb + 1) * QW)
                    tcs = rp.tile([128, QW], MMDT, tag="tcs")
                    nc.vector.tensor_tensor(
                        out=tcs[:], in0=pq[:], in1=cs_sb[:, cols], op=Mult)
                    tsn = rp.tile([128, QW], MMDT, tag="tsn")
                    nc.vector.tensor_tensor(
                        out=tsn[:], in0=pq[:], in1=sn_sb[:, cols], op=Mult)
                    pr = psP.tile([128, QW], f32, tag="proj",
                                  name=f"pr_{nm}_{hp}_{sb}")
                    nc.tensor.matmul(pr[:], rt2[:], tsn[:],
                                     start=True, stop=True)
                    yield
                    nc.vector.tensor_tensor(
                        out=dst[hp][sb][:], in0=pr[:], in1=tcs[:], op=Add)

                def gen_v(st):
                    pv = psP.tile([128, QW], f32, tag="proj",
                                  name=f"pv_{st}")
                    for kt in range(KT):
                        nc.tensor.matmul(
                            pv[:, 0:GC],
                            xsl(st // 4, kt, (st % 4) * 128, 128),
                            w_sb["wv"][:, kt * GC:(kt + 1) * GC],
                            start=(kt == 0), stop=(kt == KT - 1))
                        yield
                    vt = v_sb[st]
                    vhe = vt[:, 0:VW].rearrange("p (h e) -> p h e", e=HD + 1)
                    nc.vector.tensor_copy(
                        vhe[:, :, 0:HD],
                        pv[:, 0:GC].rearrange("p (h d) -> p h d", d=HD))

                out_tog = [0]

                def gen_out(st, db):
                    pc = psP.tile([128, QW], f32, tag="proj",
                                  name=f"pc_{st}_{db}")
                    for kt in range(2):
                        nc.tensor.matmul(
                            pc[:],
                            attnT[kt][st // 4][:, (st % 4) * 128:
                                               (st % 4) * 128 + 128],
                            wo_s[kt][:, db * QW:(db + 1) * QW],
                            start=(kt == 0), stop=(kt == 1))
                        yield
                    ob = op_.tile([128, QW], MMDT, tag="outsb")
                    dst = out[st * 128:(st + 1) * 128, db * QW:(db + 1) * QW]
                    if out_tog[0] % 2 == 0:
                        nc.vector.tensor_copy(ob[:], pc[:])
                        nc.sync.dma_start(dst, ob[:])
                    else:
                        nc.scalar.activation(ob[:], pc[:], Copy)
                        nc.scalar.dma_start(dst, ob[:])
                    out_tog[0] += 1

                def normalize(hp, qb, po, tail_out=None):
                    den16 = sp.tile([1, 2 * QW], MMDT, tag="den")
                    nc.scalar.activation(den16[:], po[HD:HD + 1, 0:2 * QW],
                                         Copy)
                    if tail_out is None:
                        # copy po values to SBUF so po frees for the next
                        # pair as soon as both copies land (not after the
                        # whole reciprocal chain)
                        vo = sp.tile([128, QW], f32, tag="vo")
                        nc.scalar.activation(vo[0:64, :], po[0:HD, 0:QW],
                                             Copy)
                        nc.scalar.activation(vo[64:128, :],
                                             po[0:HD, QW:2 * QW], Copy)
                    recb_ps = psP.tile([128, QW], f32, tag="proj",
                                       name=f"rb_{hp}_{qb}")
                    nc.tensor.matmul(recb_ps[0:64, :], ones16[0:1, 0:64],
                                     den16[0:1, 0:QW],
                                     start=True, stop=True,
                                     tile_position=(0, 0))
                    nc.tensor.matmul(recb_ps[64:128, :], ones16[0:1, 0:64],
                                     den16[0:1, QW:2 * QW],
                                     start=True, stop=True,
                                     tile_position=(0, 64))
                    recb = sp.tile([128, QW], f32, tag="recb")
                    rsc = sp.tile([128, QW], f32, tag="rscr")
                    nc.vector.reciprocal_approx_accurate(
                        out=recb[:], in_=recb_ps[:], scratch=rsc[:])
                    if tail_out is None:
                        nc.vector.tensor_tensor(
                            out=attnT[hp][qb][0:64, :],
                            in0=vo[0:64, :], in1=recb[0:64, :], op=Mult)
                        nc.vector.tensor_tensor(
                            out=attnT[hp][qb][64:128, :],
                            in0=vo[64:128, :], in1=recb[64:128, :],
                            op=Mult)
                    else:
                        # final pair: write attnT in 128-col chunks and
                        # launch the tail output projections per chunk
                        for ck in range(4):
                            cs_ = slice(ck * 128, (ck + 1) * 128)
                            nc.vector.tensor_tensor(
                                out=attnT[hp][qb][0:64, cs_],
                                in0=po[0:HD, ck * 128:(ck + 1) * 128],
                                in1=recb[0:64, cs_], op=Mult)
                            nc.vector.tensor_tensor(
                                out=attnT[hp][qb][64:128, cs_],
                                in0=po[0:HD, QW + ck * 128:
                                       QW + (ck + 1) * 128],
                                in1=recb[64:128, cs_], op=Mult)
                            for g in tail_out[ck]:
                                for _ in g:
                                    pass

                def attn_block(qb, fillers, nmicro, tail_out=None):
                    nsk = 4 * (qb + 1)
                    nsteps = 2 * nsk
                    acc = [0.0]

                    def pop(n):
                        while n > 0 and fillers:
                            try:
                                next(fillers[0])
                            except StopIteration:
                                fillers.pop(0)
                            n -= 1

                    def emit_pv(po, prts, h0, h1, kt, nsk):
                        c0 = max(0, kt * 128 - qb * QW)
                        cw = QW - c0
                        prt = prts[kt]
                        nc.tensor.matmul(
                            po[:, c0:QW],
                            v_sb[kt][:, h0 * (HD + 1):h0 * (HD + 1) + 128],
                            prt[:, 0:cw],
                            start=(kt == 0), stop=(kt == nsk - 1))
                        nc.tensor.matmul(
                            po[:, QW + c0:2 * QW],
                            v_sb[kt][:, h1 * (HD + 1):h1 * (HD + 1) + 128],
                            prt[:, QW:QW + cw],
                            start=(kt == 0), stop=(kt == nsk - 1))

                    for hp in range(2):
                        h0, h1 = 2 * hp, 2 * hp + 1
                        po = psO.tile([128, 2 * QW], f32, tag="pvacc",
                                      name=f"po_{hp}_{qb}")
                        prts = {}
                        for kt in range(nsk):
                            c0 = max(0, kt * 128 - qb * QW)
                            cw = QW - c0
                            sc = psS.tile([128, 2 * QW], f32, tag="score",
                                          name=f"sc_{hp}_{qb}_{kt}")
                            nc.tensor.matmul(
                                sc[:, 0:cw],
                                kTt[hp][kt // 4][0:64, (kt % 4) * 128:
                                                 (kt % 4) * 128 + 128],
                                qT[hp][qb][0:64, c0:QW],
                                start=True, stop=True,
                                tile_position=(0, 0))
                            nc.tensor.matmul(
                                sc[:, QW:QW + cw],
                                kTt[hp][kt // 4][64:128, (kt % 4) * 128:
                                                 (kt % 4) * 128 + 128],
                                qT[hp][qb][64:128, c0:QW],
                                start=True, stop=True,
                                tile_position=(64, 0))
                            prt = wp.tile([128, 2 * QW], MMDT, tag="probs",
                                          name=f"pr_{hp}_{qb}_{kt}")
                            prts[kt] = prt
                            diag = kt >= nsk - 4
                            if diag:
                                sc3 = sc[:].rearrange(
                                    "p (h c) -> p h c", c=QW)[:, :, 0:cw]
                                pr3 = prt[:].rearrange(
                                    "p (h c) -> p h c", c=QW)[:, :, 0:cw]
                                nc.scalar.activation(pr3, sc3, Exp,
                                                     scale=0.125)
                                pr128 = prt[:].rearrange(
                                    "p (h c) -> p h c", c=QW)[:, :, 0:128]
                                nc.gpsimd.affine_select(
                                    out=pr128, in_=pr128,
                                    pattern=[[0, 2], [1, 128]], base=0,
                                    channel_multiplier=-1,
                                    compare_op=mybir.AluOpType.is_ge,
                                    fill=0.0)
                            else:
                                nc.scalar.activation(prt[:], sc[:], Exp,
                                                     scale=0.125)
                            # filler quota while ACT runs the exp
                            acc[0] += nmicro / nsteps
                            nq = int(acc[0])
                            acc[0] -= nq
                            pop(nq)
                            if not fillers and kt >= 2:
                                # keep HAM busy: accumulate zeros onto po's
                                # garbage rows (96:127 are unused v overlap)
                                nc.tensor.matmul(
                                    po[96:128, 0:QW], ones16[0:1, 0:32],
                                    zeros16[0:1, 0:QW],
                                    start=False, stop=False,
                                    tile_position=(0, 96),
                                    skip_group_check=True)
                            if kt > 0:
                                emit_pv(po, prts, h0, h1, kt - 1, nsk)
                        emit_pv(po, prts, h0, h1, nsk - 1, nsk)
                        normalize(hp, qb, po,
                                  tail_out if qb == QB - 1 and hp == 1
                                  else None)
                    while fillers:
                        pop(1)

                # ---- block 0 projections (overlap the initial loads) -------
                for nm, dst in (("wq", qT), ("wk", kTt)):
                    for hp in range(2):
                        for _ in gen_qk_proj(nm, dst, hp, 0):
                            pass
                for st in range(4):
                    for _ in gen_v(st):
                        pass

                def proj_units(sb):
                    u = []
                    for hp in range(2):
                        u.append(gen_qk_proj("wq", qT, hp, sb))
                        u.append(gen_qk_proj("wk", kTt, hp, sb))
                    for st in range(4 * sb, 4 * sb + 4):
                        u.append(gen_v(st))
                    return u, 4 * 10 + 4 * 9

                def out_units(sb):
                    return ([gen_out(st, db)
                             for st in range(4 * sb, 4 * sb + 4)
                             for db in range(2)], 8 * 3)

                for sb in range(QB):
                    if sb + 2 < QB:
                        load_x_block(sb + 2)
                    fillers, nmicro = [], 0
                    if sb > 0:
                        ou, n = out_units(sb - 1)
                        fillers += ou
                        nmicro += n
                    if sb + 1 < QB:
                        pu, n = proj_units(sb + 1)
                        # interleave out units and proj units
                        mixed = []
                        for i in range(max(len(pu), len(fillers))):
                            if i < len(fillers):
                                mixed.append(fillers[i])
                            if i < len(pu):
                                mixed.append(pu[i])
                        fillers = mixed
                        nmicro += n
                    tail = None
                    if sb == QB - 1:
                        out_tog[0] = 0
                        tail = [[gen_out(12 + ck, 0), gen_out(12 + ck, 1)]
                                for ck in range(4)]
                    attn_block(sb, fillers, nmicro, tail)

    nc.compile()
    return nc


def _shard_inputs(x, cos, sin, wq, wk, wv, wo):
    x = np.ascontiguousarray(x, dtype=np.float32)
    cosT = np.ascontiguousarray(cos.reshape(S, HD).T, dtype=np.float32)
    sinT = np.ascontiguousarray(sin.reshape(S, HD).T, dtype=np.float32)
    cs2 = np.ascontiguousarray(np.concatenate([cosT, cosT], axis=0)
                               .astype(np.float16))
    sn2 = np.ascontiguousarray(np.concatenate([sinT, sinT], axis=0)
                               .astype(np.float16))
    f16 = np.float16

    def wdev(w):
        # [D, GC] -> [128, KT*GC] with kt chunks on columns
        return np.ascontiguousarray(
            w.reshape(KT, 128, GC).transpose(1, 0, 2).reshape(128, KT * GC))

    in_maps = []
    for c in range(NCORES):
        b, g = c // GROUPS, c % GROUPS
        rows = slice(g * GC, (g + 1) * GC)
        xT = x[b].T.astype(f16)            # [D, S]
        xdev = np.ascontiguousarray(
            xT.reshape(KT, 128, QB, QW).transpose(1, 2, 0, 3)
            .reshape(128, QB * XBW))
        in_maps.append({
            "xTd": xdev,
            "wqT": wdev(np.asarray(wq, f16)[rows, :].T),
            "wkT": wdev(np.asarray(wk, f16)[rows, :].T),
            "wvT": wdev(np.asarray(wv, f16)[rows, :].T),
            "woT": np.ascontiguousarray(np.asarray(wo, f16)[:, rows].T),
            "cs2": cs2,
            "sn2": sn2,
        })
    return in_maps


def _run(inputs, trace=False, trace_kwargs=None):
    if "nc" not in _cache:
        _cache["nc"] = _build()
    nc = _cache["nc"]
    in_maps = _shard_inputs(
        inputs["x"], inputs["cos"], inputs["sin"],
        inputs["wq"], inputs["wk"], inputs["wv"], inputs["wo"])
    res = run_bass_kernel_spmd(
        nc, in_maps, list(range(NCORES)), trace=trace,
        **(trace_kwargs or {}))
    full = np.zeros((B, S, D), dtype=np.float32)
    for c in range(NCORES):
        full[c // GROUPS] += res.results[c]["out"].astype(np.float32)
    return full, res


def kernel(**inputs):
    full, _ = _run(inputs, trace=False)
    return full
